# revision 54
# baseline (speedup 1.0000x reference)
"""Trainium2 Bass kernel for nn_ContinuousThoughtBlock.

Strategy: pure data-parallel over batch (B=8 -> 8 NeuronCores), zero
collectives.  Each core computes one batch element end-to-end:

  context = mean_L(h)                       [D]
  ctx_n   = LN(context); th0 = ctx_n @ Wagg [D]
  8 paths evolve through 4 residual-MLP steps (bf16 matmuls, weights
  stationary on the PE, activations in a [D-on-partitions, path] layout
  so LayerNorm stats become ones-vector matmuls / partition reductions)
  amps    = pruned softmax over paths (only needed after last step)
  bc      = (sum_p amps_p * th_p) @ Wbc     [D]
  gate    = sigmoid(h @ Wg + bg)            [L, D]  (fp8 DoubleRow)
  out     = LN_D(h + gate * bc)             [L, D]

V1 changes vs the earlier baseline:
  - gate matmul runs in fp8e4 DoubleRow mode (2x PE throughput): the
    host feeds h twice - an fp8 d-major copy (lhsT tiles, no device
    transposes) and the bf16 l-major copy used for context/residual/LN
  - gate stays resident in SBUF (no DRAM spill / reload)
  - h (bf16 l-major) is streamed, not resident: once for the context
    mean, once for the output phase
  - weight/act DMAs are spread across three queues (gpsimd: W1/W2,
    sync: h + Wagg/Wbc + out, scalar: fp8 gate operands)
"""

import numpy as np

import concourse.bass as bass
import concourse.mybir as mybir
import concourse.tile as tile
from concourse import bacc
from concourse.bass_utils import run_bass_kernel_spmd
from concourse.masks import make_identity

# Problem constants (hardcoded per harness contract).
B, L, D, H = 8, 2048, 1024, 4096
NUM_PATHS = 8
NUM_STEPS = 4
PRUNE = 0.1
EPS = 1e-6
KD = D // 128    # 8  D-chunks
KH = H // 128    # 32 H-chunks
ML = L // 128    # 16 L-tiles
INV_SQRT_D = 1.0 / float(np.sqrt(np.float32(D)))

F32 = mybir.dt.float32
BF16 = mybir.dt.bfloat16
F8 = mybir.dt.float8e4
AF = mybir.ActivationFunctionType
ALU = mybir.AluOpType
AX = mybir.AxisListType
DR = mybir.MatmulPerfMode.DoubleRow

WEIGHT_NAMES = [
    "input_norm_gamma", "input_norm_beta",
    "aggregator_weight", "aggregator_bias",
    "projector_norm_gamma", "projector_norm_beta",
    "projector_dense1_weight", "projector_dense1_bias",
    "projector_dense2_weight", "projector_dense2_bias",
    "broadcast_weight", "broadcast_bias",
    "gate_weight", "gate_bias",
    "output_norm_gamma", "output_norm_beta",
]


def _bc0(ap, n=128):
    """Broadcast a 1-D AP down n partitions via a stride-0 partition dim."""
    return bass.AP(tensor=ap.tensor, offset=ap.offset, ap=[[0, n]] + list(ap.ap))


def _rep0(ap, n, pos=1):
    """Insert a stride-0 free dim of extent n at position pos."""
    new = list(ap.ap)
    new.insert(pos, [0, n])
    return bass.AP(tensor=ap.tensor, offset=ap.offset, ap=new)


def build_graph(triv, debug=False):
    nc = bacc.Bacc("TRN2", target_bir_lowering=False, debug=False,
                   enable_asserts=True, num_devices=B)

    h_ext = nc.declare_dram_parameter("h_lmaj", [128, ML * D], BF16, isOutput=False)
    h8_ext = nc.declare_dram_parameter("h8", [128, ML * KD * 128], F8, isOutput=False)
    wg8_ext = nc.declare_dram_parameter("wg8", [128, KD * D], F8, isOutput=False)
    w_ext = {}
    w_ext["aggregator_weight"] = nc.declare_dram_parameter(
        "aggregator_weight", [128, KD * D], BF16, isOutput=False)
    w_ext["projector_dense1_weight"] = nc.declare_dram_parameter(
        "projector_dense1_weight", [D, H], BF16, isOutput=False)
    w_ext["projector_dense2_weight"] = nc.declare_dram_parameter(
        "projector_dense2_weight", [128, KH * D], BF16, isOutput=False)
    w_ext["broadcast_weight"] = nc.declare_dram_parameter(
        "broadcast_weight", [128, KD * D], BF16, isOutput=False)
    for n in ("input_norm_gamma", "input_norm_beta", "aggregator_bias",
              "projector_norm_gamma", "projector_norm_beta",
              "projector_dense1_bias", "projector_dense2_bias",
              "broadcast_bias", "gate_bias",
              "output_norm_gamma", "output_norm_beta"):
        shape = [H] if n == "projector_dense1_bias" else [D]
        w_ext[n] = nc.declare_dram_parameter(n, shape, F32, isOutput=False)
    out_ext = nc.declare_dram_parameter("out", [L, D], BF16, isOutput=True)

    with tile.TileContext(nc) as tc:
        _build_body(nc, tc, h_ext, h8_ext, wg8_ext, w_ext, out_ext, triv)
    nc.compile()
    return nc


def _dmajor(nc, pool, ps_pool, ident_bf, dram_ap, n, name):
    """DMA a [n*128] DRAM vector into a [128, n] d-major SBUF tile
    (tile[p, k] = v[k*128 + p]) via a bf16 [n,128] load + PE transpose."""
    rowk = pool.tile([n, 128], BF16, tag="dmaj_rowk")
    nc.gpsimd.dma_start(out=rowk[:], in_=dram_ap.rearrange("(k p) -> k p", p=128))
    ps = ps_pool.tile([128, n], BF16, tag="tr")
    nc.tensor.transpose(ps[:], rowk[:], ident_bf[0:n, 0:n])
    t = pool.tile([128, n], F32, tag=name)
    nc.scalar.copy(t[:], ps[:])
    return t


def _build_body(nc, tc, h_ext, h8_ext, wg8_ext, w, out_ext, triv):
    import contextlib
    ctx = contextlib.ExitStack()
    with ctx:
        # ---------------- pools ----------------
        singles = ctx.enter_context(tc.tile_pool(name="singles", bufs=1))
        smalls = ctx.enter_context(tc.tile_pool(name="smalls", bufs=1))
        tstate = ctx.enter_context(tc.tile_pool(name="tstate", bufs=3))
        gpool = ctx.enter_context(tc.tile_pool(name="gpool", bufs=1))
        rows = ctx.enter_context(tc.tile_pool(name="rows", bufs=1))
        hbs = ctx.enter_context(tc.tile_pool(name="hbs", bufs=2))   # streamed h pairs
        h8s = ctx.enter_context(tc.tile_pool(name="h8s", bufs=2))  # streamed fp8 h
        wg8p = ctx.enter_context(tc.tile_pool(name="wg8p", bufs=1))
        wabp = ctx.enter_context(tc.tile_pool(name="wabp", bufs=1))
        wpool = tc.alloc_tile_pool(name="wpool", bufs=1)
        dram = ctx.enter_context(tc.tile_pool(name="dram", bufs=1, space="DRAM"))

        all_triv = all(triv.values())
        ps_small = ctx.enter_context(tc.tile_pool(name="ps_small", bufs=2, space="PSUM"))
        ps_tr = ctx.enter_context(tc.tile_pool(name="ps_tr", bufs=1, space="PSUM"))
        ps_gate = ctx.enter_context(tc.tile_pool(name="ps_gate",
                                                 bufs=2 if all_triv else 1,
                                                 space="PSUM"))
        ps_ctx = ctx.enter_context(tc.tile_pool(name="ps_ctx", bufs=1, space="PSUM"))
        ps_th = ctx.enter_context(tc.tile_pool(name="ps_th", bufs=2, space="PSUM"))
        ps_y = ctx.enter_context(tc.tile_pool(name="ps_y", bufs=1, space="PSUM"))

        # ---------------- constants ----------------
        ident_bf = None
        if not all_triv:
            ident_bf = singles.tile([128, 128], BF16)
            make_identity(nc, ident_bf[:])
        ones_bf = singles.tile([128, 1], BF16)
        nc.vector.memset(ones_bf[:], 1.0)
        ones_f32 = singles.tile([128, 1], F32)
        nc.vector.memset(ones_f32[:], 1.0)
        ones_row = singles.tile([1, 128], F32)
        nc.vector.memset(ones_row[:], 1.0)
        eps1 = singles.tile([1, 1], F32)
        nc.vector.memset(eps1[:], EPS)
        eps_col = singles.tile([128, 1], F32)
        nc.vector.memset(eps_col[:], EPS)

        # resident tensors
        w1_bf = wpool.tile([128, KD, H], BF16)       # 64KB/part
        w2_bf = wpool.tile([128, KH, D], BF16)       # 64KB/part
        wab_bf = wabp.tile([128, KD, D], BF16)       # 16KB/part (Wagg, later Wbc)
        wg8 = wg8p.tile([128, KD, D], F8)            # 8KB/part
        gate_t = gpool.tile([128, ML, D], BF16)      # 32KB/part, resident

        # d-major vectors (only when nontrivial)
        gammaT_in = betaT_in = None
        if not triv["input_norm"]:
            gammaT_in = _dmajor(nc, singles, ps_tr, ident_bf,
                                w["input_norm_gamma"].ap(), KD, "g_in")
            betaT_in = _dmajor(nc, singles, ps_tr, ident_bf,
                               w["input_norm_beta"].ap(), KD, "b_in")
        gammaT_pr = betaT_pr = None
        if not triv["projector_norm"]:
            gammaT_pr = _dmajor(nc, singles, ps_tr, ident_bf,
                                w["projector_norm_gamma"].ap(), KD, "g_pr")
            betaT_pr = _dmajor(nc, singles, ps_tr, ident_bf,
                               w["projector_norm_beta"].ap(), KD, "b_pr")
        baggT = None
        if not triv["aggregator_bias"]:
            baggT = _dmajor(nc, singles, ps_tr, ident_bf,
                            w["aggregator_bias"].ap(), KD, "bagg")
        b1T = None
        if not triv["projector_dense1_bias"]:
            b1T = _dmajor(nc, singles, ps_tr, ident_bf,
                          w["projector_dense1_bias"].ap(), KH, "b1")
        b2T_rep = None
        if not triv["projector_dense2_bias"]:
            b2T = _dmajor(nc, singles, ps_tr, ident_bf,
                          w["projector_dense2_bias"].ap(), KD, "b2")
            b2T_rep = _rep0(b2T[:], NUM_PATHS, pos=2)  # [128, KD, P] view
        gbias_b = None
        if not triv["gate_bias"]:
            gbias_b = rows.tile([128, D], F32, tag="gbrow")
            nc.sync.dma_start(out=gbias_b[:], in_=_bc0(w["gate_bias"].ap()))

        # ---------------- weight DMAs (background queues) ----------------
        # gpsimd queue: W1 in H-quarters and W2 in hk-chunks, interleaved in
        # exactly the order step-1's interleaved dense1/dense2 consumes them.
        w1_src = w["projector_dense1_weight"].ap().rearrange("(k p) h -> p k h", p=128)
        w2_src = w["projector_dense2_weight"].ap().rearrange("p (k d) -> p k d", d=D)

        def w1_quarter(t):
            nc.gpsimd.dma_start(out=w1_bf[:, :, t * 1024:(t + 1) * 1024],
                                in_=w1_src[:, :, t * 1024:(t + 1) * 1024])

        def w2_chunk(c):
            nc.gpsimd.dma_start(out=w2_bf[:, 8 * c:8 * c + 8, :],
                                in_=w2_src[:, 8 * c:8 * c + 8, :])

        w1_quarter(0); w1_quarter(1); w2_chunk(0); w1_quarter(2)
        w2_chunk(1); w1_quarter(3); w2_chunk(2); w2_chunk(3)
        # scalar queue: fp8 gate operands
        nc.scalar.dma_start(out=wg8[:],
                            in_=wg8_ext.ap().rearrange("p (k d) -> p k d", d=D))
        # sync queue: Wagg (after the first ctx h pairs; needed for thought0)
        wagg_src = w["aggregator_weight"].ap().rearrange("p (k d) -> p k d", d=D)

        h8_src = h8_ext.ap().rearrange("p (m q j) -> p m q j", q=KD, j=128)
        h_src = h_ext.ap().rearrange("p (m d) -> p m d", d=D)

        # ---------------- gate matmul tiles (fp8 DoubleRow) ----------------
        def emit_gate_tiles(ms):
            for m2 in ms:  # m2 indexes pairs of L-tiles
                h8t = h8s.tile([128, 2, KD, 128], F8, tag="h8t")
                nc.scalar.dma_start(out=h8t[:], in_=h8_src[:, 2 * m2:2 * m2 + 2])
                for mm in range(2):
                    m = 2 * m2 + mm
                    for n in range(2):
                        g_ps = ps_gate.tile([128, 512], F32, tag="gps")
                        for qp in range(KD // 2):
                            nc.tensor.matmul(
                                g_ps[:],
                                h8t[:, mm, 2 * qp:2 * qp + 2, :],
                                wg8[:, 2 * qp:2 * qp + 2, n * 512:(n + 1) * 512],
                                start=(qp == 0), stop=(qp == KD // 2 - 1),
                                perf_mode=DR)
                        dst = gate_t[:, m, n * 512:(n + 1) * 512]
                        if gbias_b is not None:
                            gb = smalls.tile([128, 512], F32, tag="gbtmp")
                            nc.vector.tensor_add(gb[:], g_ps[:],
                                                 gbias_b[:, n * 512:(n + 1) * 512])
                            nc.scalar.activation(dst, gb[:], AF.Sigmoid)
                        else:
                            nc.scalar.activation(dst, g_ps[:], AF.Sigmoid)

        # ---------------- phase 1: context mean (streamed bf16 h) --------
        # 4 psum accumulation groups of 4 L-tile-pairs each, interleaved
        # with gate tile pairs so the PE never stalls long on the h stream.
        ctx_ps = ps_ctx.tile([128, 4 * KD], F32, tag="ctxps")
        emit_gate_tiles([0])

        hb_tiles = []

        def emit_ctx_group(g):
            hbts = []
            for mi in range(2):  # tile-pairs per group
                m2 = 2 * g + mi
                hbt = hbs.tile([128, 2, D], BF16, tag="hbt")
                nc.sync.dma_start(out=hbt[:], in_=h_src[:, 2 * m2:2 * m2 + 2, :])
                hbts.append(hbt)
            for k in range(KD):
                for mi in range(2):
                    for mm in range(2):
                        nc.tensor.matmul(ctx_ps[:, g * KD + k:g * KD + k + 1],
                                         hbts[mi][:, mm, k * 128:(k + 1) * 128],
                                         ones_bf[:],
                                         start=(mi == 0 and mm == 0),
                                         stop=(mi == 1 and mm == 1))

        for g in range(4):
            emit_ctx_group(g)
            if g == 1:
                nc.sync.dma_start(out=wab_bf[:], in_=wagg_src[:])
            emit_gate_tiles([g + 1])

        # merge the 4 partial context sums, then scale by 1/L
        ctxT = singles.tile([128, KD], F32)
        cview = ctx_ps[:].rearrange("p (g k) -> p k g", g=4)
        nc.vector.tensor_reduce(ctxT[:], cview, axis=AX.X, op=ALU.add)
        nc.scalar.mul(ctxT[:], ctxT[:], 1.0 / L)
        ctxT_bf = singles.tile([128, KD], BF16)
        nc.vector.tensor_copy(ctxT_bf[:], ctxT[:])

        # ---------------- phase 2: input LN + thought0 ----------------
        sqc = smalls.tile([128, KD], F32, tag="sqc")
        nc.vector.tensor_mul(sqc[:], ctxT[:], ctxT[:])
        cst_ps = ps_small.tile([1, 2 * KD], F32, tag="sm")
        nc.tensor.matmul(cst_ps[0:1, 0:KD], ones_f32[:], ctxT[:], start=True, stop=True)
        nc.tensor.matmul(cst_ps[0:1, KD:2 * KD], ones_f32[:], sqc[:], start=True, stop=True)
        csums = smalls.tile([1, 2], F32, tag="csums")
        nc.vector.tensor_reduce(csums[0:1, 0:1], cst_ps[0:1, 0:KD], axis=AX.X, op=ALU.add)
        nc.vector.tensor_reduce(csums[0:1, 1:2], cst_ps[0:1, KD:2 * KD], axis=AX.X, op=ALU.add)
        cmr = smalls.tile([1, 2], F32, tag="cmr")      # [mean, rstd]
        nc.scalar.mul(cmr[0:1, 0:1], csums[0:1, 0:1], 1.0 / D)
        csq = smalls.tile([1, 2], F32, tag="csq")
        nc.scalar.mul(csq[0:1, 0:1], csums[0:1, 1:2], 1.0 / D)   # E[x^2]
        nc.vector.tensor_mul(csq[0:1, 1:2], cmr[0:1, 0:1], cmr[0:1, 0:1])  # mean^2
        cvar = smalls.tile([1, 1], F32, tag="cvar")
        nc.vector.tensor_sub(cvar[:], csq[0:1, 0:1], csq[0:1, 1:2])
        nc.scalar.activation(cvar[:], cvar[:], AF.Sqrt, bias=eps1[0:1, :])
        nc.vector.reciprocal(cmr[0:1, 1:2], cvar[:])
        cmr_ps = ps_small.tile([128, 2], F32, tag="sm")
        nc.tensor.matmul(cmr_ps[:], ones_row[:], cmr[:], start=True, stop=True)
        cmr_b = smalls.tile([128, 2], F32, tag="cmrb")
        nc.scalar.copy(cmr_b[:], cmr_ps[:])
        ctxn = smalls.tile([128, KD], F32, tag="ctxn")
        nc.vector.tensor_scalar(ctxn[:], ctxT[:], cmr_b[:, 0:1], cmr_b[:, 1:2],
                                op0=ALU.subtract, op1=ALU.mult)
        if gammaT_in is not None:
            nc.vector.tensor_mul(ctxn[:], ctxn[:], gammaT_in[:])
            nc.vector.tensor_add(ctxn[:], ctxn[:], betaT_in[:])
        ctxn_bf = smalls.tile([128, KD], BF16, tag="ctxnbf")
        nc.vector.tensor_copy(ctxn_bf[:], ctxn[:])

        # thought0 = ctx_n @ Wagg, d-major via per-k single-instr psum groups
        th0acc = smalls.tile([128, KD], F32, tag="th0acc")
        for k in range(KD):
            thp = ps_small.tile([128, KD], F32, tag="sm")
            for dm in range(KD):
                nc.tensor.matmul(thp[:, dm:dm + 1],
                                 wab_bf[:, k, dm * 128:(dm + 1) * 128],
                                 ctxn_bf[:, k:k + 1], start=True, stop=True)
            if k == 0:
                nc.vector.tensor_copy(th0acc[:], thp[:])
            else:
                nc.vector.tensor_add(th0acc[:], th0acc[:], thp[:])
        if baggT is not None:
            nc.vector.tensor_add(th0acc[:], th0acc[:], baggT[:])
        # seed 8 paths: tT[p, k, q] = th0[k*128+p] * (1 + 0.02 q)
        tT = tstate.tile([128, KD, NUM_PATHS], F32, tag="tT")
        for q in range(NUM_PATHS):
            nc.scalar.mul(tT[:, :, q], th0acc[:], 1.0 + 0.02 * q)

        # prefetch Wbc into the shared wab buffer (WAR on th0's Wagg reads
        # is tracked by tile; the DMA lands long before the bc matmuls)
        wbc_src = w["broadcast_weight"].ap().rearrange("p (k d) -> p k d", d=D)
        nc.sync.dma_start(out=wab_bf[:], in_=wbc_src[:])

        # ---------------- phase 3: thought steps ----------------
        _gate_slices = {0: [5], 1: [6], 2: [7], 3: []}
        for step in range(NUM_STEPS):
            if step in _gate_slices:
                emit_gate_tiles(_gate_slices[step])
            last = step == NUM_STEPS - 1
            sq = smalls.tile([128, KD, NUM_PATHS], F32, tag="sq")
            nc.vector.tensor_mul(sq[:], tT[:], tT[:])
            st_ps = ps_small.tile([1, 128], F32, tag="sm")
            nc.tensor.matmul(st_ps[0:1, 0:64], ones_f32[:],
                             tT[:].rearrange("a k q -> a q k"), start=True, stop=True)
            nc.tensor.matmul(st_ps[0:1, 64:128], ones_f32[:],
                             sq[:].rearrange("a k q -> a q k"), start=True, stop=True)
            ms = smalls.tile([1, 2 * NUM_PATHS], F32, tag="ms")  # [mean | msq->rstd]
            sums16 = smalls.tile([1, 2 * NUM_PATHS], F32, tag="sums")
            nc.vector.tensor_reduce(sums16[0:1, :],
                                    st_ps[0:1, :].rearrange("a (g k) -> a g k", k=KD),
                                    axis=AX.X, op=ALU.add)
            nc.scalar.mul(ms[0:1, :], sums16[0:1, :], 1.0 / D)
            m2v = smalls.tile([1, NUM_PATHS], F32, tag="m2")
            nc.vector.tensor_mul(m2v[0:1, :], ms[0:1, 0:NUM_PATHS], ms[0:1, 0:NUM_PATHS])
            var = smalls.tile([1, NUM_PATHS], F32, tag="var")
            nc.vector.tensor_sub(var[0:1, :], ms[0:1, NUM_PATHS:], m2v[0:1, :])
            nc.scalar.activation(var[0:1, :], var[0:1, :], AF.Sqrt, bias=eps1[0:1, :])
            nc.vector.reciprocal(ms[0:1, NUM_PATHS:], var[0:1, :])
            mr_ps = ps_small.tile([128, 2 * NUM_PATHS], F32, tag="sm")
            nc.tensor.matmul(mr_ps[:], ones_row[:], ms[:], start=True, stop=True)
            mr_b = mr_ps

            # normalize all (k, q) at once with stride-0 broadcasts
            tn_bf = smalls.tile([128, KD, NUM_PATHS], BF16, tag="tnbf")
            tc_f = smalls.tile([128, KD, NUM_PATHS], F32, tag="tcf")
            nc.vector.tensor_tensor(out=tc_f[:], in0=tT[:],
                                    in1=_rep0(mr_b[:, 0:NUM_PATHS], KD),
                                    op=ALU.subtract)
            if gammaT_pr is not None:
                nc.vector.tensor_tensor(out=tc_f[:], in0=tc_f[:],
                                        in1=_rep0(mr_b[:, NUM_PATHS:], KD),
                                        op=ALU.mult)
                nc.vector.tensor_tensor(out=tc_f[:], in0=tc_f[:],
                                        in1=_rep0(gammaT_pr[:], NUM_PATHS, pos=2),
                                        op=ALU.mult)
                nc.vector.tensor_tensor(out=tn_bf[:], in0=tc_f[:],
                                        in1=_rep0(betaT_pr[:], NUM_PATHS, pos=2),
                                        op=ALU.add)
            else:
                nc.vector.tensor_tensor(out=tn_bf[:], in0=tc_f[:],
                                        in1=_rep0(mr_b[:, NUM_PATHS:], KD),
                                        op=ALU.mult)

            # dense1 + dense2 interleaved per 8-H-chunk block: d2 of block t
            # runs right after d1 of block t+1, consuming W1/W2 chunks in
            # their DMA arrival order so step 1 streams behind the weights.
            y_ps = ps_y.tile([128, 4, KD * NUM_PATHS], F32, tag="yps")
            x1_tiles = []

            def emit_d1_block(tblk):
                x1_bf_t = tstate.tile([128, 8, NUM_PATHS], BF16, tag=f"x1_{tblk}")
                x1_tiles.append(x1_bf_t)
                x1_ps = ps_th.tile([128, 8 * NUM_PATHS], F32, tag="th")
                for hs in range(8):
                    mh = tblk * 8 + hs
                    for k in range(KD):
                        nc.tensor.matmul(x1_ps[:, hs * 8:(hs + 1) * 8],
                                         w1_bf[:, k, mh * 128:(mh + 1) * 128],
                                         tn_bf[:, k, :],
                                         start=(k == 0), stop=(k == KD - 1))
                # tanh-gelu (matches jax.nn.gelu approximate=True)
                xs = smalls.tile([128, 8 * NUM_PATHS], F32, tag="gelu_x")
                if b1T is not None:
                    for hs in range(8):
                        mh = tblk * 8 + hs
                        nc.scalar.activation(xs[:, hs * 8:(hs + 1) * 8],
                                             x1_ps[:, hs * 8:(hs + 1) * 8],
                                             AF.Identity, bias=b1T[:, mh:mh + 1])
                else:
                    nc.scalar.copy(xs[:], x1_ps[:])
                u = smalls.tile([128, 8 * NUM_PATHS], F32, tag="gelu_u")
                nc.vector.tensor_mul(u[:], xs[:], xs[:])
                nc.vector.tensor_mul(u[:], u[:], xs[:])
                nc.vector.scalar_tensor_tensor(u[:], u[:], 0.044715, xs[:],
                                               op0=ALU.mult, op1=ALU.add)
                nc.scalar.activation(u[:], u[:], AF.Tanh, scale=0.7978845608028654)
                nc.vector.scalar_tensor_tensor(u[:], u[:], 1.0, xs[:],
                                               op0=ALU.add, op1=ALU.mult)
                nc.scalar.mul(x1_bf_t[:].rearrange("a b c -> a (b c)"), u[:], 0.5)

            def emit_d2_block(tblk):
                # contiguous psum group per (tblk, dm) column: partial sums
                # over this block's 8 hk chunks, combined after all blocks
                for dm in range(KD):
                    for hs in range(8):
                        nc.tensor.matmul(y_ps[:, tblk, dm * 8:(dm + 1) * 8],
                                         w2_bf[:, tblk * 8 + hs,
                                               dm * 128:(dm + 1) * 128],
                                         x1_tiles[tblk][:, hs, :],
                                         start=(hs == 0), stop=(hs == 7))

            emit_d1_block(0)
            emit_d1_block(1)
            emit_d2_block(0)
            emit_d1_block(2)
            emit_d2_block(1)
            emit_d1_block(3)
            emit_d2_block(2)
            emit_d2_block(3)
            y_sb = smalls.tile([128, KD * NUM_PATHS], F32, tag="ysb")
            nc.vector.tensor_reduce(y_sb[:],
                                    y_ps[:].rearrange("a g c -> a c g"),
                                    axis=AX.X, op=ALU.add)
            tT_new = tstate.tile([128, KD, NUM_PATHS], F32, tag="tT")
            yv = y_sb[:].rearrange("a (k q) -> a k q", k=KD)
            if b2T_rep is not None:
                nc.vector.tensor_add(tT_new[:], yv, b2T_rep)
                nc.vector.tensor_add(tT_new[:], tT_new[:], tT[:])
            else:
                nc.vector.tensor_add(tT_new[:], yv, tT[:])
            tT = tT_new

            if last:
                tT_bf = smalls.tile([128, KD, NUM_PATHS], BF16, tag="tTbf")
                nc.vector.tensor_copy(tT_bf[:], tT[:])
                sc_ps = ps_small.tile([1, NUM_PATHS], F32, tag="sm")
                for k in range(KD):
                    nc.tensor.matmul(sc_ps[:], ctxT_bf[:, k:k + 1], tT_bf[:, k, :],
                                     start=(k == 0), stop=(k == KD - 1))
                sc = smalls.tile([1, NUM_PATHS], F32, tag="sc")
                nc.scalar.mul(sc[:], sc_ps[:], INV_SQRT_D)
                negmax = smalls.tile([1, 1], F32, tag="negmax")
                nc.vector.tensor_reduce(negmax[:], sc[:], axis=AX.X, op=ALU.max,
                                        negate=True)
                esum = smalls.tile([1, 1], F32, tag="esum")
                ex = smalls.tile([1, NUM_PATHS], F32, tag="ex")
                nc.scalar.activation(ex[:], sc[:], AF.Exp, bias=negmax[0:1, :],
                                     accum_out=esum[:])
                rsum = smalls.tile([1, 1], F32, tag="rsum")
                nc.vector.reciprocal(rsum[:], esum[:])
                amps0 = smalls.tile([1, NUM_PATHS], F32, tag="amps0")
                nc.vector.tensor_scalar(amps0[:], ex[:], rsum[0:1, :], None, op0=ALU.mult)
                mask = smalls.tile([1, NUM_PATHS], F32, tag="mask")
                nc.vector.tensor_scalar(mask[:], amps0[:], PRUNE, None, op0=ALU.is_ge)
                pruned = smalls.tile([1, NUM_PATHS], F32, tag="pruned")
                nc.vector.tensor_mul(pruned[:], amps0[:], mask[:])
                psum_s = smalls.tile([1, 1], F32, tag="psums")
                nc.vector.tensor_reduce(psum_s[:], pruned[:], axis=AX.X, op=ALU.add)
                nc.vector.tensor_scalar(psum_s[:], psum_s[:], EPS, None, op0=ALU.add)
                rr = smalls.tile([1, 1], F32, tag="rr")
                nc.vector.reciprocal(rr[:], psum_s[:])
                ampsF = smalls.tile([1, NUM_PATHS], F32, tag="ampsF")
                nc.vector.tensor_scalar(ampsF[:], pruned[:], rr[0:1, :], None, op0=ALU.mult)

        # W1/W2 are dead after the last step: free the space and prefetch
        # the epilogue's h pairs across the queues while bc is computed.
        wpool.release()
        hb2p = ctx.enter_context(tc.tile_pool(name="hb2p", bufs=1))
        hb_all = hb2p.tile([128, ML, D], BF16)
        for i, q in enumerate((nc.gpsimd, nc.sync, nc.scalar, nc.gpsimd)):
            q.dma_start(out=hb_all[:, 4 * i:4 * i + 4, :],
                        in_=h_src[:, 4 * i:4 * i + 4, :])

        # ---------------- phase 4: collapse + bc ----------------
        ab_ps = ps_small.tile([128, NUM_PATHS], F32, tag="sm")
        nc.tensor.matmul(ab_ps[:], ones_row[:], ampsF[:], start=True, stop=True)
        amps_sb = smalls.tile([128, NUM_PATHS], F32, tag="ampssb")
        nc.scalar.copy(amps_sb[:], ab_ps[:])
        prod = smalls.tile([128, KD, NUM_PATHS], F32, tag="prod")
        nc.vector.tensor_tensor(out=prod[:], in0=tT[:], in1=_rep0(amps_sb[:], KD),
                                op=ALU.mult)
        finalT = smalls.tile([128, KD], F32, tag="finalT")
        nc.vector.tensor_reduce(finalT[:], prod[:], axis=AX.X, op=ALU.add)
        finalT_bf = smalls.tile([128, KD], BF16, tag="finalTbf")
        nc.vector.tensor_copy(finalT_bf[:], finalT[:])

        # bc row [1, D] in two 512-halves, then K=1 matmul broadcast
        bc_bf = singles.tile([128, D], BF16)
        for n in range(2):
            bc_ps = ps_gate.tile([1, 512], F32, tag="gps")
            for k in range(KD):
                nc.tensor.matmul(bc_ps[:],
                                 finalT_bf[:, k:k + 1],
                                 wab_bf[:, k, n * 512:(n + 1) * 512],
                                 start=(k == 0), stop=(k == KD - 1))
            bc_half = rows.tile([1, 512], F32, tag="row")
            if not triv["broadcast_bias"]:
                bbh = rows.tile([1, 512], F32, tag="brow")
                nc.sync.dma_start(
                    out=bbh[:],
                    in_=w["broadcast_bias"].ap()[n * 512:(n + 1) * 512]
                        .rearrange("(a d) -> a d", a=1))
                nc.vector.tensor_add(bc_half[:], bc_ps[:], bbh[:])
            else:
                nc.scalar.copy(bc_half[:], bc_ps[:])
            bcb_ps = ps_gate.tile([128, 512], F32, tag="gps")
            nc.tensor.matmul(bcb_ps[:], ones_row[:], bc_half[:], start=True, stop=True)
            nc.scalar.copy(bc_bf[:, n * 512:(n + 1) * 512], bcb_ps[:])

        gin = ctx.enter_context(tc.tile_pool(name="gin", bufs=5))
        fin = ctx.enter_context(tc.tile_pool(name="fin", bufs=6))
        gamma_out_b = beta_out_b = None
        if not triv["output_norm"]:
            fin1 = ctx.enter_context(tc.tile_pool(name="fin1", bufs=1))
            gamma_out_b = fin1.tile([128, D], F32)
            nc.sync.dma_start(out=gamma_out_b[:], in_=_bc0(w["output_norm_gamma"].ap()))
            beta_out_b = fin1.tile([128, D], F32)
            nc.sync.dma_start(out=beta_out_b[:], in_=_bc0(w["output_norm_beta"].ap()))

        # ---------------- phase 5: final LN + output ----------------
        # h prefetched in SBUF; gate read straight from SBUF.  gpsimd
        # pre-multiplies gate*bc for the first four pairs while the
        # vector engine starts on the other four; the normalize runs on
        # the scalar engine (per-partition scale/bias APs) so vector and
        # scalar split the full-width passes roughly evenly.
        # bc materialized as a unit-stride pair so the DVE multiply gets
        # its fast mode (stride-0 broadcasts fall back to 1x)
        bc2 = gin.tile([128, 2, D], BF16, tag="bc2")
        nc.vector.tensor_copy(bc2[:, 0, :], bc_bf[:])
        nc.vector.tensor_copy(bc2[:, 1, :], bc_bf[:])
        p2_tiles = {}
        for pr in range(5):
            # gpsimd owns the gate*bc multiply for pairs 0-4 (processed last)
            p2 = gin.tile([128, 2, D], BF16, tag="p1")
            nc.gpsimd.tensor_tensor(out=p2[:], in0=gate_t[:, 2 * pr:2 * pr + 2, :],
                                    in1=bc2[:], op=ALU.mult)
            p2_tiles[pr] = p2
        for m in list(range(10, ML)) + list(range(10)):
            pr = m // 2
            if m % 2 == 0 and pr >= 5:
                p2 = gin.tile([128, 2, D], BF16, tag="p1v")
                nc.vector.tensor_tensor(
                    out=p2[:].rearrange("a b c -> a (b c)"),
                    in0=gate_t[:, m:m + 2, :].rearrange("a b c -> a (b c)"),
                    in1=bc2[:].rearrange("a b c -> a (b c)"), op=ALU.mult)
                p2_tiles[pr] = p2
            p2 = p2_tiles[pr]
            # pre = p1 + h, with fused row-sum accumulation
            pre = fin.tile([128, D], BF16, tag="pre")
            if m % 2 == 0:
                rs = fin.tile([128, 2, 2], F32, tag="rs")  # [m%2][sum, sumsq]
                p2_tiles[(pr, "rs")] = rs
            rs = p2_tiles[(pr, "rs")]
            nc.vector.scalar_tensor_tensor(pre[:], p2[:, m % 2, :], 1.0,
                                           hb_all[:, m, :],
                                           op0=ALU.mult, op1=ALU.add,
                                           accum_out=rs[:, m % 2, 0:1])
            sqs = fin.tile([128, D], BF16, tag="sqs")
            nc.scalar.activation(sqs[:], pre[:], AF.Square,
                                 accum_out=rs[:, m % 2, 1:2])
            p2_tiles[(pr, m % 2)] = pre
            if m % 2 == 1:
                # pair-batched stats: mean/rstd for both rows in few ops
                mv = fin.tile([128, 2, 2], F32, tag="mv")  # [m][mean, E[x^2]]
                nc.vector.tensor_scalar(mv[:], rs[:], 1.0 / D, None, op0=ALU.mult)
                var2 = fin.tile([128, 2], F32, tag="var2")
                nc.vector.tensor_tensor(out=var2[:], in0=mv[:, :, 0],
                                        in1=mv[:, :, 0], op=ALU.mult)
                nc.vector.tensor_sub(var2[:], mv[:, :, 1], var2[:])
                sd2 = fin.tile([128, 2], F32, tag="sd2")
                nc.scalar.activation(sd2[:], var2[:], AF.Sqrt, bias=eps_col[:])
                rstd2 = fin.tile([128, 2], F32, tag="rstd2")
                nc.vector.reciprocal(rstd2[:], sd2[:])
                nm2 = fin.tile([128, 2], F32, tag="nm2")
                nc.vector.scalar_tensor_tensor(nm2[:], mv[:, :, 0], -1.0,
                                               rstd2[:], op0=ALU.mult,
                                               op1=ALU.mult)
                for mm in range(2):
                    mo = m - 1 + mm
                    preo = p2_tiles[(pr, mm)]
                    o = fin.tile([128, D], BF16, tag="o")
                    # normalize on ACT: out = pre*rstd + (-mean*rstd)
                    nc.scalar.activation(o[:], preo[:], AF.Identity,
                                         bias=nm2[:, mm:mm + 1],
                                         scale=rstd2[:, mm:mm + 1])
                    if gamma_out_b is not None:
                        nc.vector.tensor_mul(o[:], o[:], gamma_out_b[:])
                        nc.vector.tensor_add(o[:], o[:], beta_out_b[:])
                    nc.sync.dma_start(out=out_ext.ap()[mo * 128:(mo + 1) * 128, :],
                                      in_=o[:])


def _triv_flags(inputs):
    def ones(x):
        return bool(np.all(np.asarray(x) == 1.0))

    def zeros(x):
        return bool(np.all(np.asarray(x) == 0.0))

    return {
        "input_norm": ones(inputs["input_norm_gamma"]) and zeros(inputs["input_norm_beta"]),
        "projector_norm": ones(inputs["projector_norm_gamma"]) and zeros(inputs["projector_norm_beta"]),
        "output_norm": ones(inputs["output_norm_gamma"]) and zeros(inputs["output_norm_beta"]),
        "aggregator_bias": zeros(inputs["aggregator_bias"]),
        "projector_dense1_bias": zeros(inputs["projector_dense1_bias"]),
        "projector_dense2_bias": zeros(inputs["projector_dense2_bias"]),
        "broadcast_bias": zeros(inputs["broadcast_bias"]),
        "gate_bias": zeros(inputs["gate_bias"]),
    }


_GRAPH_CACHE = {}

BF16_INPUTS = ("aggregator_weight", "projector_dense1_weight",
               "projector_dense2_weight", "broadcast_weight")


def prep_in_maps(inputs):
    """Build per-core input maps; big tensors are converted to bf16/fp8 and
    repacked partition-major on the host so every DMA descriptor is one
    large contiguous read per partition row."""
    import ml_dtypes

    def pmajor(a):
        k = a.shape[0] // 128
        return np.ascontiguousarray(
            a.reshape(k, 128, -1).transpose(1, 0, 2).reshape(128, -1))

    hs_f32 = np.asarray(inputs["hidden_states"], dtype=np.float32)
    hs = np.ascontiguousarray(hs_f32.astype(ml_dtypes.bfloat16))
    hs8 = np.ascontiguousarray(hs_f32.astype(ml_dtypes.float8_e4m3))
    assert hs.shape == (B, L, D)
    weights = {}
    for n in WEIGHT_NAMES:
        a = np.asarray(inputs[n], dtype=np.float32)
        if n in BF16_INPUTS:
            a = a.astype(ml_dtypes.bfloat16)
        weights[n] = np.ascontiguousarray(a)
    for n in ("aggregator_weight", "broadcast_weight", "projector_dense2_weight"):
        weights[n] = pmajor(weights[n])
    wg8 = pmajor(np.asarray(inputs["gate_weight"], dtype=np.float32)
                 .astype(ml_dtypes.float8_e4m3))
    del weights["gate_weight"]
    in_maps = []
    for b in range(B):
        # h8 packed [p][m][q][j]: h8[p, m, q, j] = h[m*128+j, q*128+p]
        h8 = np.ascontiguousarray(
            hs8[b].reshape(ML, 128, KD, 128).transpose(3, 0, 2, 1).reshape(128, -1))
        m = {"h_lmaj": pmajor(hs[b]), "h8": h8, "wg8": wg8}
        m.update(weights)
        in_maps.append(m)
    return in_maps


def kernel(**inputs):
    triv = _triv_flags(inputs)
    key = tuple(sorted(triv.items()))
    if key not in _GRAPH_CACHE:
        _GRAPH_CACHE[key] = build_graph(triv)
    nc = _GRAPH_CACHE[key]
    in_maps = prep_in_maps(inputs)
    res = run_bass_kernel_spmd(nc, in_maps, core_ids=list(range(B)))
    out = np.stack([np.asarray(res.results[b]["out"]).astype(np.float32)
                    for b in range(B)], axis=0)
    return out


# revision 55
# speedup vs baseline: 1.0483x; 1.0483x over previous
"""Trainium2 Bass kernel for nn_ContinuousThoughtBlock.

Strategy: pure data-parallel over batch (B=8 -> 8 NeuronCores), zero
collectives.  Each core computes one batch element end-to-end:

  context = mean_L(h)                       [D]
  ctx_n   = LN(context); th0 = ctx_n @ Wagg [D]
  8 paths evolve through 4 residual-MLP steps (bf16 matmuls, weights
  stationary on the PE, activations in a [D-on-partitions, path] layout
  so LayerNorm stats become ones-vector matmuls / partition reductions)
  amps    = pruned softmax over paths (only needed after last step)
  bc      = (sum_p amps_p * th_p) @ Wbc     [D]
  gate    = sigmoid(h @ Wg + bg)            [L, D]  (fp8 DoubleRow)
  out     = LN_D(h + gate * bc)             [L, D]

V1 changes vs the earlier baseline:
  - gate matmul runs in fp8e4 DoubleRow mode (2x PE throughput): the
    host feeds h twice - an fp8 d-major copy (lhsT tiles, no device
    transposes) and the bf16 l-major copy used for context/residual/LN
  - gate stays resident in SBUF (no DRAM spill / reload)
  - h (bf16 l-major) is streamed, not resident: once for the context
    mean, once for the output phase
  - weight/act DMAs are spread across three queues (gpsimd: W1/W2,
    sync: h + Wagg/Wbc + out, scalar: fp8 gate operands)
"""

import numpy as np

import concourse.bass as bass
import concourse.mybir as mybir
import concourse.tile as tile
from concourse import bacc
from concourse.bass_utils import run_bass_kernel_spmd
from concourse.masks import make_identity

# Problem constants (hardcoded per harness contract).
B, L, D, H = 8, 2048, 1024, 4096
NUM_PATHS = 8
NUM_STEPS = 4
PRUNE = 0.1
EPS = 1e-6
KD = D // 128    # 8  D-chunks
KH = H // 128    # 32 H-chunks
ML = L // 128    # 16 L-tiles
INV_SQRT_D = 1.0 / float(np.sqrt(np.float32(D)))

F32 = mybir.dt.float32
BF16 = mybir.dt.bfloat16
F8 = mybir.dt.float8e4
AF = mybir.ActivationFunctionType
ALU = mybir.AluOpType
AX = mybir.AxisListType
DR = mybir.MatmulPerfMode.DoubleRow

WEIGHT_NAMES = [
    "input_norm_gamma", "input_norm_beta",
    "aggregator_weight", "aggregator_bias",
    "projector_norm_gamma", "projector_norm_beta",
    "projector_dense1_weight", "projector_dense1_bias",
    "projector_dense2_weight", "projector_dense2_bias",
    "broadcast_weight", "broadcast_bias",
    "gate_weight", "gate_bias",
    "output_norm_gamma", "output_norm_beta",
]


def _bc0(ap, n=128):
    """Broadcast a 1-D AP down n partitions via a stride-0 partition dim."""
    return bass.AP(tensor=ap.tensor, offset=ap.offset, ap=[[0, n]] + list(ap.ap))


def _rep0(ap, n, pos=1):
    """Insert a stride-0 free dim of extent n at position pos."""
    new = list(ap.ap)
    new.insert(pos, [0, n])
    return bass.AP(tensor=ap.tensor, offset=ap.offset, ap=new)


def build_graph(triv, debug=False):
    nc = bacc.Bacc("TRN2", target_bir_lowering=False, debug=False,
                   enable_asserts=True, num_devices=B)

    h_ext = nc.declare_dram_parameter("h_lmaj", [128, ML * D], BF16, isOutput=False)
    h8_ext = nc.declare_dram_parameter("h8", [128, ML * KD * 128], F8, isOutput=False)
    wg8_ext = nc.declare_dram_parameter("wg8", [128, KD * D], F8, isOutput=False)
    w_ext = {}
    w_ext["aggregator_weight"] = nc.declare_dram_parameter(
        "aggregator_weight", [128, KD * D], BF16, isOutput=False)
    w_ext["projector_dense1_weight"] = nc.declare_dram_parameter(
        "projector_dense1_weight", [D, H], BF16, isOutput=False)
    w_ext["projector_dense2_weight"] = nc.declare_dram_parameter(
        "projector_dense2_weight", [128, KH * D], BF16, isOutput=False)
    w_ext["broadcast_weight"] = nc.declare_dram_parameter(
        "broadcast_weight", [128, KD * D], BF16, isOutput=False)
    for n in ("input_norm_gamma", "input_norm_beta", "aggregator_bias",
              "projector_norm_gamma", "projector_norm_beta",
              "projector_dense1_bias", "projector_dense2_bias",
              "broadcast_bias", "gate_bias",
              "output_norm_gamma", "output_norm_beta"):
        shape = [H] if n == "projector_dense1_bias" else [D]
        w_ext[n] = nc.declare_dram_parameter(n, shape, F32, isOutput=False)
    out_ext = nc.declare_dram_parameter("out", [L, D], BF16, isOutput=True)

    with tile.TileContext(nc) as tc:
        _build_body(nc, tc, h_ext, h8_ext, wg8_ext, w_ext, out_ext, triv)
    nc.compile()
    return nc


def _dmajor(nc, pool, ps_pool, ident_bf, dram_ap, n, name):
    """DMA a [n*128] DRAM vector into a [128, n] d-major SBUF tile
    (tile[p, k] = v[k*128 + p]) via a bf16 [n,128] load + PE transpose."""
    rowk = pool.tile([n, 128], BF16, tag="dmaj_rowk")
    nc.gpsimd.dma_start(out=rowk[:], in_=dram_ap.rearrange("(k p) -> k p", p=128))
    ps = ps_pool.tile([128, n], BF16, tag="tr")
    nc.tensor.transpose(ps[:], rowk[:], ident_bf[0:n, 0:n])
    t = pool.tile([128, n], F32, tag=name)
    nc.scalar.copy(t[:], ps[:])
    return t


def _build_body(nc, tc, h_ext, h8_ext, wg8_ext, w, out_ext, triv):
    import contextlib
    ctx = contextlib.ExitStack()
    with ctx:
        # ---------------- pools ----------------
        singles = ctx.enter_context(tc.tile_pool(name="singles", bufs=1))
        smalls = ctx.enter_context(tc.tile_pool(name="smalls", bufs=1))
        tstate = ctx.enter_context(tc.tile_pool(name="tstate", bufs=3))
        gpool = ctx.enter_context(tc.tile_pool(name="gpool", bufs=1))
        rows = ctx.enter_context(tc.tile_pool(name="rows", bufs=1))
        hbs = ctx.enter_context(tc.tile_pool(name="hbs", bufs=2))   # streamed h pairs
        h8s = ctx.enter_context(tc.tile_pool(name="h8s", bufs=2))  # streamed fp8 h
        wg8p = ctx.enter_context(tc.tile_pool(name="wg8p", bufs=1))
        wabp = ctx.enter_context(tc.tile_pool(name="wabp", bufs=1))
        wpool = tc.alloc_tile_pool(name="wpool", bufs=1)
        dram = ctx.enter_context(tc.tile_pool(name="dram", bufs=1, space="DRAM"))

        all_triv = all(triv.values())
        ps_small = ctx.enter_context(tc.tile_pool(name="ps_small", bufs=2, space="PSUM"))
        ps_tr = ctx.enter_context(tc.tile_pool(name="ps_tr", bufs=1, space="PSUM"))
        ps_gate = ctx.enter_context(tc.tile_pool(name="ps_gate",
                                                 bufs=2 if all_triv else 1,
                                                 space="PSUM"))
        ps_ctx = ctx.enter_context(tc.tile_pool(name="ps_ctx", bufs=1, space="PSUM"))
        ps_th = ctx.enter_context(tc.tile_pool(name="ps_th", bufs=2, space="PSUM"))
        ps_y = ctx.enter_context(tc.tile_pool(name="ps_y", bufs=1, space="PSUM"))

        # ---------------- constants ----------------
        ident_bf = None
        if not all_triv:
            ident_bf = singles.tile([128, 128], BF16)
            make_identity(nc, ident_bf[:])
        ones_bf = singles.tile([128, 1], BF16)
        nc.vector.memset(ones_bf[:], 1.0)
        ones_f32 = singles.tile([128, 1], F32)
        nc.vector.memset(ones_f32[:], 1.0)
        ones_row = singles.tile([1, 128], F32)
        nc.vector.memset(ones_row[:], 1.0)
        eps1 = singles.tile([1, 1], F32)
        nc.vector.memset(eps1[:], EPS)
        eps_col = singles.tile([128, 1], F32)
        nc.vector.memset(eps_col[:], EPS)

        # resident tensors
        w1_bf = wpool.tile([128, KD, H], BF16)       # 64KB/part
        w2_bf = wpool.tile([128, KH, D], BF16)       # 64KB/part
        wab_bf = wabp.tile([128, KD, D], BF16)       # 16KB/part (Wagg, later Wbc)
        wg8 = wg8p.tile([128, KD, D], F8)            # 8KB/part
        gate_t = gpool.tile([128, ML, D], BF16)      # 32KB/part, resident

        # d-major vectors (only when nontrivial)
        gammaT_in = betaT_in = None
        if not triv["input_norm"]:
            gammaT_in = _dmajor(nc, singles, ps_tr, ident_bf,
                                w["input_norm_gamma"].ap(), KD, "g_in")
            betaT_in = _dmajor(nc, singles, ps_tr, ident_bf,
                               w["input_norm_beta"].ap(), KD, "b_in")
        gammaT_pr = betaT_pr = None
        if not triv["projector_norm"]:
            gammaT_pr = _dmajor(nc, singles, ps_tr, ident_bf,
                                w["projector_norm_gamma"].ap(), KD, "g_pr")
            betaT_pr = _dmajor(nc, singles, ps_tr, ident_bf,
                               w["projector_norm_beta"].ap(), KD, "b_pr")
        baggT = None
        if not triv["aggregator_bias"]:
            baggT = _dmajor(nc, singles, ps_tr, ident_bf,
                            w["aggregator_bias"].ap(), KD, "bagg")
        b1T = None
        if not triv["projector_dense1_bias"]:
            b1T = _dmajor(nc, singles, ps_tr, ident_bf,
                          w["projector_dense1_bias"].ap(), KH, "b1")
        b2T_rep = None
        if not triv["projector_dense2_bias"]:
            b2T = _dmajor(nc, singles, ps_tr, ident_bf,
                          w["projector_dense2_bias"].ap(), KD, "b2")
            b2T_rep = _rep0(b2T[:], NUM_PATHS, pos=2)  # [128, KD, P] view
        gbias_b = None
        if not triv["gate_bias"]:
            gbias_b = rows.tile([128, D], F32, tag="gbrow")
            nc.sync.dma_start(out=gbias_b[:], in_=_bc0(w["gate_bias"].ap()))

        # ---------------- weight DMAs (background queues) ----------------
        # gpsimd queue: W1 in H-quarters and W2 in hk-chunks, interleaved in
        # exactly the order step-1's interleaved dense1/dense2 consumes them.
        w1_src = w["projector_dense1_weight"].ap().rearrange("(k p) h -> p k h", p=128)
        w2_src = w["projector_dense2_weight"].ap().rearrange("p (k d) -> p k d", d=D)

        def w1_quarter(t):
            nc.gpsimd.dma_start(out=w1_bf[:, :, t * 1024:(t + 1) * 1024],
                                in_=w1_src[:, :, t * 1024:(t + 1) * 1024])

        def w2_chunk(c):
            nc.gpsimd.dma_start(out=w2_bf[:, 8 * c:8 * c + 8, :],
                                in_=w2_src[:, 8 * c:8 * c + 8, :])

        w1_quarter(0); w1_quarter(1); w2_chunk(0); w1_quarter(2)
        w2_chunk(1); w1_quarter(3); w2_chunk(2); w2_chunk(3)
        # scalar queue: fp8 gate operands
        nc.scalar.dma_start(out=wg8[:],
                            in_=wg8_ext.ap().rearrange("p (k d) -> p k d", d=D))
        # sync queue: Wagg (after the first ctx h pairs; needed for thought0)
        wagg_src = w["aggregator_weight"].ap().rearrange("p (k d) -> p k d", d=D)

        h8_src = h8_ext.ap().rearrange("p (m q j) -> p m q j", q=KD, j=128)
        h_src = h_ext.ap().rearrange("p (m d) -> p m d", d=D)

        # ---------------- gate matmul tiles (fp8 DoubleRow) ----------------
        def emit_gate_tiles(ms):
            for m2 in ms:  # m2 indexes pairs of L-tiles
                h8t = h8s.tile([128, 2, KD, 128], F8, tag="h8t")
                nc.scalar.dma_start(out=h8t[:], in_=h8_src[:, 2 * m2:2 * m2 + 2])
                for mm in range(2):
                    m = 2 * m2 + mm
                    for n in range(2):
                        g_ps = ps_gate.tile([128, 512], F32, tag="gps")
                        for qp in range(KD // 2):
                            nc.tensor.matmul(
                                g_ps[:],
                                h8t[:, mm, 2 * qp:2 * qp + 2, :],
                                wg8[:, 2 * qp:2 * qp + 2, n * 512:(n + 1) * 512],
                                start=(qp == 0), stop=(qp == KD // 2 - 1),
                                perf_mode=DR)
                        dst = gate_t[:, m, n * 512:(n + 1) * 512]
                        if gbias_b is not None:
                            gb = smalls.tile([128, 512], F32, tag="gbtmp")
                            nc.vector.tensor_add(gb[:], g_ps[:],
                                                 gbias_b[:, n * 512:(n + 1) * 512])
                            nc.scalar.activation(dst, gb[:], AF.Sigmoid)
                        else:
                            nc.scalar.activation(dst, g_ps[:], AF.Sigmoid)

        # ---------------- phase 1: context mean (streamed bf16 h) --------
        # 4 psum accumulation groups of 4 L-tile-pairs each, interleaved
        # with gate tile pairs so the PE never stalls long on the h stream.
        ctx_ps = ps_ctx.tile([128, 4 * KD], F32, tag="ctxps")
        emit_gate_tiles([0])

        hb_tiles = []

        def emit_ctx_group(g):
            hbts = []
            for mi in range(2):  # tile-pairs per group
                m2 = 2 * g + mi
                hbt = hbs.tile([128, 2, D], BF16, tag="hbt")
                nc.sync.dma_start(out=hbt[:], in_=h_src[:, 2 * m2:2 * m2 + 2, :])
                hbts.append(hbt)
            for k in range(KD):
                for mi in range(2):
                    for mm in range(2):
                        nc.tensor.matmul(ctx_ps[:, g * KD + k:g * KD + k + 1],
                                         hbts[mi][:, mm, k * 128:(k + 1) * 128],
                                         ones_bf[:],
                                         start=(mi == 0 and mm == 0),
                                         stop=(mi == 1 and mm == 1))

        for g in range(4):
            emit_ctx_group(g)
            if g == 1:
                nc.sync.dma_start(out=wab_bf[:], in_=wagg_src[:])
            emit_gate_tiles([g + 1])

        # merge the 4 partial context sums, then scale by 1/L
        ctxT = singles.tile([128, KD], F32)
        cview = ctx_ps[:].rearrange("p (g k) -> p k g", g=4)
        nc.vector.tensor_reduce(ctxT[:], cview, axis=AX.X, op=ALU.add)
        nc.scalar.mul(ctxT[:], ctxT[:], 1.0 / L)
        ctxT_bf = singles.tile([128, KD], BF16)
        nc.vector.tensor_copy(ctxT_bf[:], ctxT[:])

        # ---------------- phase 2: input LN + thought0 ----------------
        sqc = smalls.tile([128, KD], F32, tag="sqc")
        nc.vector.tensor_mul(sqc[:], ctxT[:], ctxT[:])
        cst_ps = ps_small.tile([1, 2 * KD], F32, tag="sm")
        nc.tensor.matmul(cst_ps[0:1, 0:KD], ones_f32[:], ctxT[:], start=True, stop=True)
        nc.tensor.matmul(cst_ps[0:1, KD:2 * KD], ones_f32[:], sqc[:], start=True, stop=True)
        csums = smalls.tile([1, 2], F32, tag="csums")
        nc.vector.tensor_reduce(csums[0:1, 0:1], cst_ps[0:1, 0:KD], axis=AX.X, op=ALU.add)
        nc.vector.tensor_reduce(csums[0:1, 1:2], cst_ps[0:1, KD:2 * KD], axis=AX.X, op=ALU.add)
        cmr = smalls.tile([1, 2], F32, tag="cmr")      # [mean, rstd]
        nc.scalar.mul(cmr[0:1, 0:1], csums[0:1, 0:1], 1.0 / D)
        csq = smalls.tile([1, 2], F32, tag="csq")
        nc.scalar.mul(csq[0:1, 0:1], csums[0:1, 1:2], 1.0 / D)   # E[x^2]
        nc.vector.tensor_mul(csq[0:1, 1:2], cmr[0:1, 0:1], cmr[0:1, 0:1])  # mean^2
        cvar = smalls.tile([1, 1], F32, tag="cvar")
        nc.vector.tensor_sub(cvar[:], csq[0:1, 0:1], csq[0:1, 1:2])
        nc.scalar.activation(cvar[:], cvar[:], AF.Sqrt, bias=eps1[0:1, :])
        nc.vector.reciprocal(cmr[0:1, 1:2], cvar[:])
        cmr_ps = ps_small.tile([128, 2], F32, tag="sm")
        nc.tensor.matmul(cmr_ps[:], ones_row[:], cmr[:], start=True, stop=True)
        cmr_b = smalls.tile([128, 2], F32, tag="cmrb")
        nc.scalar.copy(cmr_b[:], cmr_ps[:])
        ctxn = smalls.tile([128, KD], F32, tag="ctxn")
        nc.vector.tensor_scalar(ctxn[:], ctxT[:], cmr_b[:, 0:1], cmr_b[:, 1:2],
                                op0=ALU.subtract, op1=ALU.mult)
        if gammaT_in is not None:
            nc.vector.tensor_mul(ctxn[:], ctxn[:], gammaT_in[:])
            nc.vector.tensor_add(ctxn[:], ctxn[:], betaT_in[:])
        ctxn_bf = smalls.tile([128, KD], BF16, tag="ctxnbf")
        nc.vector.tensor_copy(ctxn_bf[:], ctxn[:])

        # thought0 = ctx_n @ Wagg, d-major via per-k single-instr psum groups
        th0acc = smalls.tile([128, KD], F32, tag="th0acc")
        for k in range(KD):
            thp = ps_small.tile([128, KD], F32, tag="sm")
            for dm in range(KD):
                nc.tensor.matmul(thp[:, dm:dm + 1],
                                 wab_bf[:, k, dm * 128:(dm + 1) * 128],
                                 ctxn_bf[:, k:k + 1], start=True, stop=True)
            if k == 0:
                nc.vector.tensor_copy(th0acc[:], thp[:])
            else:
                nc.vector.tensor_add(th0acc[:], th0acc[:], thp[:])
        if baggT is not None:
            nc.vector.tensor_add(th0acc[:], th0acc[:], baggT[:])
        # seed 8 paths: tT[p, k, q] = th0[k*128+p] * (1 + 0.02 q)
        tT = tstate.tile([128, KD, NUM_PATHS], F32, tag="tT")
        for q in range(NUM_PATHS):
            nc.scalar.mul(tT[:, :, q], th0acc[:], 1.0 + 0.02 * q)

        # prefetch Wbc into the shared wab buffer (WAR on th0's Wagg reads
        # is tracked by tile; the DMA lands long before the bc matmuls)
        wbc_src = w["broadcast_weight"].ap().rearrange("p (k d) -> p k d", d=D)
        nc.sync.dma_start(out=wab_bf[:], in_=wbc_src[:])

        # ---------------- phase 3: thought steps ----------------
        _gate_slices = {0: [5], 1: [6], 2: [7], 3: []}
        for step in range(NUM_STEPS):
            if step in _gate_slices:
                emit_gate_tiles(_gate_slices[step])
            last = step == NUM_STEPS - 1
            sq = smalls.tile([128, KD, NUM_PATHS], F32, tag="sq")
            nc.vector.tensor_mul(sq[:], tT[:], tT[:])
            st_ps = ps_small.tile([1, 128], F32, tag="sm")
            nc.tensor.matmul(st_ps[0:1, 0:64], ones_f32[:],
                             tT[:].rearrange("a k q -> a q k"), start=True, stop=True)
            nc.tensor.matmul(st_ps[0:1, 64:128], ones_f32[:],
                             sq[:].rearrange("a k q -> a q k"), start=True, stop=True)
            ms = smalls.tile([1, 2 * NUM_PATHS], F32, tag="ms")  # [mean | msq->rstd]
            sums16 = smalls.tile([1, 2 * NUM_PATHS], F32, tag="sums")
            nc.vector.tensor_reduce(sums16[0:1, :],
                                    st_ps[0:1, :].rearrange("a (g k) -> a g k", k=KD),
                                    axis=AX.X, op=ALU.add)
            nc.scalar.mul(ms[0:1, :], sums16[0:1, :], 1.0 / D)
            m2v = smalls.tile([1, NUM_PATHS], F32, tag="m2")
            nc.vector.tensor_mul(m2v[0:1, :], ms[0:1, 0:NUM_PATHS], ms[0:1, 0:NUM_PATHS])
            var = smalls.tile([1, NUM_PATHS], F32, tag="var")
            nc.vector.tensor_sub(var[0:1, :], ms[0:1, NUM_PATHS:], m2v[0:1, :])
            nc.scalar.activation(var[0:1, :], var[0:1, :], AF.Sqrt, bias=eps1[0:1, :])
            nc.vector.reciprocal(ms[0:1, NUM_PATHS:], var[0:1, :])
            mr_ps = ps_small.tile([128, 2 * NUM_PATHS], F32, tag="sm")
            nc.tensor.matmul(mr_ps[:], ones_row[:], ms[:], start=True, stop=True)
            mr_b = mr_ps

            # normalize all (k, q) at once with stride-0 broadcasts
            tn_bf = smalls.tile([128, KD, NUM_PATHS], BF16, tag="tnbf")
            tc_f = smalls.tile([128, KD, NUM_PATHS], F32, tag="tcf")
            nc.vector.tensor_tensor(out=tc_f[:], in0=tT[:],
                                    in1=_rep0(mr_b[:, 0:NUM_PATHS], KD),
                                    op=ALU.subtract)
            if gammaT_pr is not None:
                nc.vector.tensor_tensor(out=tc_f[:], in0=tc_f[:],
                                        in1=_rep0(mr_b[:, NUM_PATHS:], KD),
                                        op=ALU.mult)
                nc.vector.tensor_tensor(out=tc_f[:], in0=tc_f[:],
                                        in1=_rep0(gammaT_pr[:], NUM_PATHS, pos=2),
                                        op=ALU.mult)
                nc.vector.tensor_tensor(out=tn_bf[:], in0=tc_f[:],
                                        in1=_rep0(betaT_pr[:], NUM_PATHS, pos=2),
                                        op=ALU.add)
            else:
                nc.vector.tensor_tensor(out=tn_bf[:], in0=tc_f[:],
                                        in1=_rep0(mr_b[:, NUM_PATHS:], KD),
                                        op=ALU.mult)

            # dense1 + dense2 interleaved per 8-H-chunk block: d2 of block t
            # runs right after d1 of block t+1, consuming W1/W2 chunks in
            # their DMA arrival order so step 1 streams behind the weights.
            y_ps = ps_y.tile([128, 4, KD * NUM_PATHS], F32, tag="yps")
            x1_tiles = []

            def emit_d1_block(tblk):
                x1_bf_t = tstate.tile([128, 8, NUM_PATHS], BF16, tag=f"x1_{tblk}")
                x1_tiles.append(x1_bf_t)
                x1_ps = ps_th.tile([128, 8 * NUM_PATHS], F32, tag="th")
                for hs in range(8):
                    mh = tblk * 8 + hs
                    for k in range(KD):
                        nc.tensor.matmul(x1_ps[:, hs * 8:(hs + 1) * 8],
                                         w1_bf[:, k, mh * 128:(mh + 1) * 128],
                                         tn_bf[:, k, :],
                                         start=(k == 0), stop=(k == KD - 1))
                # tanh-gelu (matches jax.nn.gelu approximate=True)
                xs = smalls.tile([128, 8 * NUM_PATHS], F32, tag="gelu_x")
                if b1T is not None:
                    for hs in range(8):
                        mh = tblk * 8 + hs
                        nc.scalar.activation(xs[:, hs * 8:(hs + 1) * 8],
                                             x1_ps[:, hs * 8:(hs + 1) * 8],
                                             AF.Identity, bias=b1T[:, mh:mh + 1])
                else:
                    nc.scalar.copy(xs[:], x1_ps[:])
                u = smalls.tile([128, 8 * NUM_PATHS], F32, tag="gelu_u")
                nc.vector.tensor_mul(u[:], xs[:], xs[:])
                nc.vector.tensor_mul(u[:], u[:], xs[:])
                nc.vector.scalar_tensor_tensor(u[:], u[:], 0.044715, xs[:],
                                               op0=ALU.mult, op1=ALU.add)
                nc.scalar.activation(u[:], u[:], AF.Tanh, scale=0.7978845608028654)
                nc.vector.scalar_tensor_tensor(u[:], u[:], 1.0, xs[:],
                                               op0=ALU.add, op1=ALU.mult)
                nc.scalar.mul(x1_bf_t[:].rearrange("a b c -> a (b c)"), u[:], 0.5)

            def emit_d2_block(tblk):
                # contiguous psum group per (tblk, dm) column: partial sums
                # over this block's 8 hk chunks, combined after all blocks
                for dm in range(KD):
                    for hs in range(8):
                        nc.tensor.matmul(y_ps[:, tblk, dm * 8:(dm + 1) * 8],
                                         w2_bf[:, tblk * 8 + hs,
                                               dm * 128:(dm + 1) * 128],
                                         x1_tiles[tblk][:, hs, :],
                                         start=(hs == 0), stop=(hs == 7))

            emit_d1_block(0)
            emit_d1_block(1)
            emit_d2_block(0)
            emit_d1_block(2)
            emit_d2_block(1)
            emit_d1_block(3)
            emit_d2_block(2)
            emit_d2_block(3)
            y_sb = smalls.tile([128, KD * NUM_PATHS], F32, tag="ysb")
            nc.vector.tensor_reduce(y_sb[:],
                                    y_ps[:].rearrange("a g c -> a c g"),
                                    axis=AX.X, op=ALU.add)
            tT_new = tstate.tile([128, KD, NUM_PATHS], F32, tag="tT")
            yv = y_sb[:].rearrange("a (k q) -> a k q", k=KD)
            if b2T_rep is not None:
                nc.vector.tensor_add(tT_new[:], yv, b2T_rep)
                nc.vector.tensor_add(tT_new[:], tT_new[:], tT[:])
            else:
                nc.vector.tensor_add(tT_new[:], yv, tT[:])
            tT = tT_new

            if last:
                tT_bf = smalls.tile([128, KD, NUM_PATHS], BF16, tag="tTbf")
                nc.vector.tensor_copy(tT_bf[:], tT[:])
                sc_ps = ps_small.tile([1, NUM_PATHS], F32, tag="sm")
                for k in range(KD):
                    nc.tensor.matmul(sc_ps[:], ctxT_bf[:, k:k + 1], tT_bf[:, k, :],
                                     start=(k == 0), stop=(k == KD - 1))
                sc = smalls.tile([1, NUM_PATHS], F32, tag="sc")
                nc.scalar.mul(sc[:], sc_ps[:], INV_SQRT_D)
                negmax = smalls.tile([1, 1], F32, tag="negmax")
                nc.vector.tensor_reduce(negmax[:], sc[:], axis=AX.X, op=ALU.max,
                                        negate=True)
                esum = smalls.tile([1, 1], F32, tag="esum")
                ex = smalls.tile([1, NUM_PATHS], F32, tag="ex")
                nc.scalar.activation(ex[:], sc[:], AF.Exp, bias=negmax[0:1, :],
                                     accum_out=esum[:])
                rsum = smalls.tile([1, 1], F32, tag="rsum")
                nc.vector.reciprocal(rsum[:], esum[:])
                amps0 = smalls.tile([1, NUM_PATHS], F32, tag="amps0")
                nc.vector.tensor_scalar(amps0[:], ex[:], rsum[0:1, :], None, op0=ALU.mult)
                mask = smalls.tile([1, NUM_PATHS], F32, tag="mask")
                nc.vector.tensor_scalar(mask[:], amps0[:], PRUNE, None, op0=ALU.is_ge)
                pruned = smalls.tile([1, NUM_PATHS], F32, tag="pruned")
                nc.vector.tensor_mul(pruned[:], amps0[:], mask[:])
                psum_s = smalls.tile([1, 1], F32, tag="psums")
                nc.vector.tensor_reduce(psum_s[:], pruned[:], axis=AX.X, op=ALU.add)
                nc.vector.tensor_scalar(psum_s[:], psum_s[:], EPS, None, op0=ALU.add)
                rr = smalls.tile([1, 1], F32, tag="rr")
                nc.vector.reciprocal(rr[:], psum_s[:])
                ampsF = smalls.tile([1, NUM_PATHS], F32, tag="ampsF")
                nc.vector.tensor_scalar(ampsF[:], pruned[:], rr[0:1, :], None, op0=ALU.mult)

        # W1/W2 are dead after the last step: free the space and prefetch
        # the epilogue's h pairs across the queues while bc is computed.
        wpool.release()
        hb2p = ctx.enter_context(tc.tile_pool(name="hb2p", bufs=1))
        hb_all = hb2p.tile([128, ML, D], BF16)
        for i, q in enumerate((nc.gpsimd, nc.sync, nc.scalar, nc.gpsimd)):
            q.dma_start(out=hb_all[:, 4 * i:4 * i + 4, :],
                        in_=h_src[:, 4 * i:4 * i + 4, :])

        # ---------------- phase 4: collapse + bc ----------------
        ab_ps = ps_small.tile([128, NUM_PATHS], F32, tag="sm")
        nc.tensor.matmul(ab_ps[:], ones_row[:], ampsF[:], start=True, stop=True)
        amps_sb = smalls.tile([128, NUM_PATHS], F32, tag="ampssb")
        nc.scalar.copy(amps_sb[:], ab_ps[:])
        prod = smalls.tile([128, KD, NUM_PATHS], F32, tag="prod")
        nc.vector.tensor_tensor(out=prod[:], in0=tT[:], in1=_rep0(amps_sb[:], KD),
                                op=ALU.mult)
        finalT = smalls.tile([128, KD], F32, tag="finalT")
        nc.vector.tensor_reduce(finalT[:], prod[:], axis=AX.X, op=ALU.add)
        finalT_bf = smalls.tile([128, KD], BF16, tag="finalTbf")
        nc.vector.tensor_copy(finalT_bf[:], finalT[:])

        # bc row [1, D] in two 512-halves, then K=1 matmul broadcast
        bc_bf = singles.tile([128, D], BF16)
        for n in range(2):
            bc_ps = ps_gate.tile([1, 512], F32, tag="gps")
            for k in range(KD):
                nc.tensor.matmul(bc_ps[:],
                                 finalT_bf[:, k:k + 1],
                                 wab_bf[:, k, n * 512:(n + 1) * 512],
                                 start=(k == 0), stop=(k == KD - 1))
            bc_half = rows.tile([1, 512], F32, tag="row")
            if not triv["broadcast_bias"]:
                bbh = rows.tile([1, 512], F32, tag="brow")
                nc.sync.dma_start(
                    out=bbh[:],
                    in_=w["broadcast_bias"].ap()[n * 512:(n + 1) * 512]
                        .rearrange("(a d) -> a d", a=1))
                nc.vector.tensor_add(bc_half[:], bc_ps[:], bbh[:])
            else:
                nc.scalar.copy(bc_half[:], bc_ps[:])
            bcb_ps = ps_gate.tile([128, 512], F32, tag="gps")
            nc.tensor.matmul(bcb_ps[:], ones_row[:], bc_half[:], start=True, stop=True)
            nc.scalar.copy(bc_bf[:, n * 512:(n + 1) * 512], bcb_ps[:])

        gin = ctx.enter_context(tc.tile_pool(name="gin", bufs=4))
        fin = ctx.enter_context(tc.tile_pool(name="fin", bufs=6))
        gamma_out_b = beta_out_b = None
        if not triv["output_norm"]:
            fin1 = ctx.enter_context(tc.tile_pool(name="fin1", bufs=1))
            gamma_out_b = fin1.tile([128, D], F32)
            nc.sync.dma_start(out=gamma_out_b[:], in_=_bc0(w["output_norm_gamma"].ap()))
            beta_out_b = fin1.tile([128, D], F32)
            nc.sync.dma_start(out=beta_out_b[:], in_=_bc0(w["output_norm_beta"].ap()))

        # ---------------- phase 5: final LN + output ----------------
        # h prefetched in SBUF; gate read straight from SBUF.  gpsimd
        # pre-multiplies gate*bc for the first four pairs while the
        # vector engine starts on the other four; the normalize runs on
        # the scalar engine (per-partition scale/bias APs) so vector and
        # scalar split the full-width passes roughly evenly.
        # bc materialized as a unit-stride pair so the DVE multiply gets
        # its fast mode (stride-0 broadcasts fall back to 1x)
        bc2 = gin.tile([128, 2, D], BF16, tag="bc2")
        nc.vector.tensor_copy(bc2[:, 0, :], bc_bf[:])
        nc.vector.tensor_copy(bc2[:, 1, :], bc_bf[:])
        p2_tiles = {}
        for pr in range(4):
            # gpsimd owns the gate*bc multiply for pairs 0-3 (processed last)
            p2 = gin.tile([128, 2, D], BF16, tag="p1")
            nc.gpsimd.tensor_tensor(out=p2[:], in0=gate_t[:, 2 * pr:2 * pr + 2, :],
                                    in1=bc2[:], op=ALU.mult)
            p2_tiles[pr] = p2
        for m in list(range(8, ML)) + list(range(8)):
            pr = m // 2
            if m % 2 == 0 and pr >= 4:
                p2 = gin.tile([128, 2, D], BF16, tag="p1v")
                nc.vector.tensor_tensor(
                    out=p2[:].rearrange("a b c -> a (b c)"),
                    in0=gate_t[:, m:m + 2, :].rearrange("a b c -> a (b c)"),
                    in1=bc2[:].rearrange("a b c -> a (b c)"), op=ALU.mult)
                p2_tiles[pr] = p2
            p2 = p2_tiles[pr]
            # pre = p1 + h, with fused row-sum accumulation
            pre = fin.tile([128, D], BF16, tag="pre")
            if m % 2 == 0:
                rs = fin.tile([128, 2, 2], F32, tag="rs")  # [m%2][sum, sumsq]
                p2_tiles[(pr, "rs")] = rs
            rs = p2_tiles[(pr, "rs")]
            nc.vector.scalar_tensor_tensor(pre[:], p2[:, m % 2, :], 1.0,
                                           hb_all[:, m, :],
                                           op0=ALU.mult, op1=ALU.add,
                                           accum_out=rs[:, m % 2, 0:1])
            sqs = fin.tile([128, D], BF16, tag="sqs")
            nc.scalar.activation(sqs[:], pre[:], AF.Square,
                                 accum_out=rs[:, m % 2, 1:2])
            p2_tiles[(pr, m % 2)] = pre
            if m % 2 == 1:
                # pair-batched stats: mean/rstd for both rows in few ops
                mv = fin.tile([128, 2, 2], F32, tag="mv")  # [m][mean, E[x^2]]
                nc.vector.tensor_scalar(mv[:], rs[:], 1.0 / D, None, op0=ALU.mult)
                var2 = fin.tile([128, 2], F32, tag="var2")
                nc.vector.tensor_tensor(out=var2[:], in0=mv[:, :, 0],
                                        in1=mv[:, :, 0], op=ALU.mult)
                nc.vector.tensor_sub(var2[:], mv[:, :, 1], var2[:])
                sd2 = fin.tile([128, 2], F32, tag="sd2")
                nc.scalar.activation(sd2[:], var2[:], AF.Sqrt, bias=eps_col[:])
                rstd2 = fin.tile([128, 2], F32, tag="rstd2")
                nc.vector.reciprocal(rstd2[:], sd2[:])
                nm2 = fin.tile([128, 2], F32, tag="nm2")
                nc.vector.scalar_tensor_tensor(nm2[:], mv[:, :, 0], -1.0,
                                               rstd2[:], op0=ALU.mult,
                                               op1=ALU.mult)
                for mm in range(2):
                    mo = m - 1 + mm
                    preo = p2_tiles[(pr, mm)]
                    o = fin.tile([128, D], BF16, tag="o")
                    nc.vector.tensor_scalar(o[:], preo[:], mv[:, mm, 0:1],
                                            rstd2[:, mm:mm + 1],
                                            op0=ALU.subtract, op1=ALU.mult)
                    if gamma_out_b is not None:
                        nc.vector.tensor_mul(o[:], o[:], gamma_out_b[:])
                        nc.vector.tensor_add(o[:], o[:], beta_out_b[:])
                    nc.sync.dma_start(out=out_ext.ap()[mo * 128:(mo + 1) * 128, :],
                                      in_=o[:])


def _triv_flags(inputs):
    def ones(x):
        return bool(np.all(np.asarray(x) == 1.0))

    def zeros(x):
        return bool(np.all(np.asarray(x) == 0.0))

    return {
        "input_norm": ones(inputs["input_norm_gamma"]) and zeros(inputs["input_norm_beta"]),
        "projector_norm": ones(inputs["projector_norm_gamma"]) and zeros(inputs["projector_norm_beta"]),
        "output_norm": ones(inputs["output_norm_gamma"]) and zeros(inputs["output_norm_beta"]),
        "aggregator_bias": zeros(inputs["aggregator_bias"]),
        "projector_dense1_bias": zeros(inputs["projector_dense1_bias"]),
        "projector_dense2_bias": zeros(inputs["projector_dense2_bias"]),
        "broadcast_bias": zeros(inputs["broadcast_bias"]),
        "gate_bias": zeros(inputs["gate_bias"]),
    }


_GRAPH_CACHE = {}

BF16_INPUTS = ("aggregator_weight", "projector_dense1_weight",
               "projector_dense2_weight", "broadcast_weight")


def prep_in_maps(inputs):
    """Build per-core input maps; big tensors are converted to bf16/fp8 and
    repacked partition-major on the host so every DMA descriptor is one
    large contiguous read per partition row."""
    import ml_dtypes

    def pmajor(a):
        k = a.shape[0] // 128
        return np.ascontiguousarray(
            a.reshape(k, 128, -1).transpose(1, 0, 2).reshape(128, -1))

    hs_f32 = np.asarray(inputs["hidden_states"], dtype=np.float32)
    hs = np.ascontiguousarray(hs_f32.astype(ml_dtypes.bfloat16))
    hs8 = np.ascontiguousarray(hs_f32.astype(ml_dtypes.float8_e4m3))
    assert hs.shape == (B, L, D)
    weights = {}
    for n in WEIGHT_NAMES:
        a = np.asarray(inputs[n], dtype=np.float32)
        if n in BF16_INPUTS:
            a = a.astype(ml_dtypes.bfloat16)
        weights[n] = np.ascontiguousarray(a)
    for n in ("aggregator_weight", "broadcast_weight", "projector_dense2_weight"):
        weights[n] = pmajor(weights[n])
    wg8 = pmajor(np.asarray(inputs["gate_weight"], dtype=np.float32)
                 .astype(ml_dtypes.float8_e4m3))
    del weights["gate_weight"]
    in_maps = []
    for b in range(B):
        # h8 packed [p][m][q][j]: h8[p, m, q, j] = h[m*128+j, q*128+p]
        h8 = np.ascontiguousarray(
            hs8[b].reshape(ML, 128, KD, 128).transpose(3, 0, 2, 1).reshape(128, -1))
        m = {"h_lmaj": pmajor(hs[b]), "h8": h8, "wg8": wg8}
        m.update(weights)
        in_maps.append(m)
    return in_maps


def kernel(**inputs):
    triv = _triv_flags(inputs)
    key = tuple(sorted(triv.items()))
    if key not in _GRAPH_CACHE:
        _GRAPH_CACHE[key] = build_graph(triv)
    nc = _GRAPH_CACHE[key]
    in_maps = prep_in_maps(inputs)
    res = run_bass_kernel_spmd(nc, in_maps, core_ids=list(range(B)))
    out = np.stack([np.asarray(res.results[b]["out"]).astype(np.float32)
                    for b in range(B)], axis=0)
    return out
